# revision 1
# baseline (speedup 1.0000x reference)
"""KGE (TransR-style) loss kernel for Trainium2, 8 NeuronCores.

Strategy:
  - Host: sort the M=8192 triples by relation id (pure index manipulation),
    pad each relation's segment to 128-row blocks -> ~96 single-relation
    blocks, distributed evenly across the 8 cores (same block count per
    core, so one SPMD program serves all cores). Per-core relation tables
    (W blocks, r rows) are sharded host-side per the block list.
  - Device (per core, per block b):
      * three indirect DMAs gather the h/pos/neg entity rows into
        X = [H | P | N]  (128 x 384)   [GPSIMD/SWDGE]
      * D_pos = H - P, D_neg = H - N; squares + row reductions  [DVE]
      * PE transpose D -> D^T; ACT copies PSUM->SBUF
      * matmul D^T.T @ W_b accumulated with a K=NB one-hot matmul adding
        r_b -> (h - t) @ W + r in PSUM  [PE]
      * score diff col stored per block; softplus tail batched over all
        blocks at the end (2 act-table loads total instead of ~2/block)
  - reg = 0.5*sum(X^2) per row, masked+scaled by 1e-5 via the wval input;
    relation-embedding reg via per-block counts.
  - Final: free-dim reduce + ones-matmul partition reduce -> one f32 per
    core; host sums the 8 partials and divides by M.
"""

import os
from contextlib import ExitStack

import numpy as np

import concourse.bass as bass
import concourse.tile as tile
from concourse import bacc, mybir
from concourse.masks import make_identity

M = 8192
E = 128
N_ENT = 500000
N_REL = 64
LAM = 1e-5
P = 128
N_CORES = 8
PAD_BIAS = -30000.0

f32 = mybir.dt.float32
i32 = mybir.dt.int32

_cache = {}


def _build(NB: int):
    """Build + compile the single-core SPMD program for NB blocks/core."""
    nc = bacc.Bacc(
        "TRN2",
        target_bir_lowering=False,
        debug=False,
        num_devices=N_CORES,
    )

    ent = nc.dram_tensor("ent", (N_ENT, E), f32, kind="ExternalInput").ap()
    idx3 = nc.dram_tensor("idx3", (P, NB * 3), i32, kind="ExternalInput").ap()
    mbias = nc.dram_tensor("mbias", (P, NB), f32, kind="ExternalInput").ap()
    wval = nc.dram_tensor("wval", (P, NB), f32, kind="ExternalInput").ap()
    w_all = nc.dram_tensor("w_all", (P, NB * P), f32, kind="ExternalInput").ap()
    r_in = nc.dram_tensor("r_blk", (NB, E), f32, kind="ExternalInput").ap()
    lsel = nc.dram_tensor("lsel", (NB, NB * P), f32, kind="ExternalInput").ap()
    cnt = nc.dram_tensor("cnt", (NB, 1), f32, kind="ExternalInput").ap()
    out = nc.dram_tensor("out", (1, 1), f32, kind="ExternalOutput").ap()

    with tile.TileContext(nc) as tc, ExitStack() as ctx:
        const = ctx.enter_context(tc.tile_pool(name="const", bufs=1))
        xp = ctx.enter_context(tc.tile_pool(name="xp", bufs=6))
        dp = ctx.enter_context(tc.tile_pool(name="dp", bufs=3))
        dtp = ctx.enter_context(tc.tile_pool(name="dtp", bufs=3))
        scrp = ctx.enter_context(tc.tile_pool(name="scrp", bufs=3))
        colp = ctx.enter_context(tc.tile_pool(name="colp", bufs=4))
        ps_t = ctx.enter_context(tc.tile_pool(name="ps_t", bufs=2, space="PSUM"))
        ps_mm = ctx.enter_context(tc.tile_pool(name="ps_mm", bufs=2, space="PSUM"))

        # constants / small inputs
        iden = const.tile([P, P], f32)
        make_identity(nc, iden[:])
        ones_col = const.tile([P, 1], f32)
        nc.gpsimd.memset(ones_col[:], 1.0)

        idx3_sb = const.tile([P, NB * 3], i32)
        nc.sync.dma_start(out=idx3_sb[:], in_=idx3[:])
        mb_sb = const.tile([P, NB], f32)
        nc.sync.dma_start(out=mb_sb[:], in_=mbias[:])
        wv_sb = const.tile([P, NB], f32)
        nc.sync.dma_start(out=wv_sb[:], in_=wval[:])
        cnt_sb = const.tile([NB, 1], f32)
        nc.sync.dma_start(out=cnt_sb[:], in_=cnt[:])
        w_sb = const.tile([P, NB * P], f32)
        nc.sync.dma_start(out=w_sb[:], in_=w_all[:])
        r_blk = const.tile([NB, E], f32)
        nc.sync.dma_start(out=r_blk[:], in_=r_in[:])
        lsel_sb = const.tile([NB, NB * P], f32)
        nc.sync.dma_start(out=lsel_sb[:], in_=lsel[:])

        # per-block score-diff columns and raw reg columns
        dcols = const.tile([P, NB], f32)
        regs = const.tile([P, NB], f32)

        for b in range(NB):
            # three gathers: hardware indirect DMA takes one index per
            # partition and reads out.free_size contiguous elems from it
            x = xp.tile([P, 3 * E], f32, tag="x")
            for j in range(3):
                nc.gpsimd.indirect_dma_start(
                    out=x[:, j * E : (j + 1) * E],
                    out_offset=None,
                    in_=ent[:],
                    in_offset=bass.IndirectOffsetOnAxis(
                        ap=idx3_sb[:, 3 * b + j : 3 * b + j + 1], axis=0
                    ),
                )

            # raw reg col: sum over [H|P|N] of squares (mask+scale at tail);
            # ACT Square with accum_out frees the DVE for score work
            xsq = scrp.tile([P, 3 * E], f32, tag="xsq")
            nc.scalar.activation(
                out=xsq[:], in_=x[:],
                func=mybir.ActivationFunctionType.Square,
                accum_out=regs[:, b : b + 1],
            )

            # D_pos = H - P, D_neg = H - N
            d_pos = dp.tile([P, E], f32, tag="dpos")
            nc.vector.tensor_tensor(
                out=d_pos[:], in0=x[:, 0:E], in1=x[:, E : 2 * E],
                op=mybir.AluOpType.subtract,
            )
            d_neg = dp.tile([P, E], f32, tag="dneg")
            nc.vector.tensor_tensor(
                out=d_neg[:], in0=x[:, 0:E], in1=x[:, 2 * E : 3 * E],
                op=mybir.AluOpType.subtract,
            )

            # transpose D -> D^T (PSUM), copy to SBUF on ACT
            dpt_ps = ps_t.tile([P, P], f32, tag="tp")
            nc.tensor.transpose(out=dpt_ps[:], in_=d_pos[:], identity=iden[:])
            dnt_ps = ps_t.tile([P, P], f32, tag="tn")
            nc.tensor.transpose(out=dnt_ps[:], in_=d_neg[:], identity=iden[:])
            dpt = dtp.tile([P, P], f32, tag="dpt")
            nc.scalar.copy(dpt[:], dpt_ps[:])
            dnt = dtp.tile([P, P], f32, tag="dnt")
            nc.scalar.copy(dnt[:], dnt_ps[:])

            # (h - t) @ W + r
            wb = w_sb[:, b * P : (b + 1) * P]
            lb = lsel_sb[:, b * P : (b + 1) * P]
            pos_ps = ps_mm.tile([P, E], f32, tag="mp")
            nc.tensor.matmul(out=pos_ps[:], lhsT=dpt[:], rhs=wb, start=True, stop=False)
            nc.tensor.matmul(out=pos_ps[:], lhsT=lb, rhs=r_blk[:], start=False, stop=True)
            neg_ps = ps_mm.tile([P, E], f32, tag="mn")
            nc.tensor.matmul(out=neg_ps[:], lhsT=dnt[:], rhs=wb, start=True, stop=False)
            nc.tensor.matmul(out=neg_ps[:], lhsT=lb, rhs=r_blk[:], start=False, stop=True)

            # score diff col (x2): sum(neg^2) - sum(pos^2); ACT Square reads
            # PSUM (DVE cannot read two PSUM inputs) and fuses the reduction
            psq = scrp.tile([P, E], f32, tag="psq")
            spos = colp.tile([P, 1], f32, tag="sp")
            nc.scalar.activation(
                out=psq[:], in_=pos_ps[:],
                func=mybir.ActivationFunctionType.Square,
                accum_out=spos[:],
            )
            nsq = scrp.tile([P, E], f32, tag="nsq")
            sneg = colp.tile([P, 1], f32, tag="sn")
            nc.scalar.activation(
                out=nsq[:], in_=neg_ps[:],
                func=mybir.ActivationFunctionType.Square,
                accum_out=sneg[:],
            )
            nc.vector.tensor_tensor(
                out=dcols[:, b : b + 1], in0=sneg[:], in1=spos[:],
                op=mybir.AluOpType.subtract,
            )

        # ---- batched tail over all NB blocks ----
        # loss = softplus(0.5*dcols + mbias) = relu(y) + ln(1 + exp(-|y|))
        dm = const.tile([P, NB], f32)
        nc.vector.tensor_scalar_mul(out=dm[:], in0=dcols[:], scalar1=0.5)
        nc.vector.tensor_tensor(
            out=dm[:], in0=dm[:], in1=mb_sb[:], op=mybir.AluOpType.add
        )
        t_abs = const.tile([P, NB], f32)
        nc.scalar.activation(
            out=t_abs[:], in_=dm[:], func=mybir.ActivationFunctionType.Abs
        )
        t_exp = const.tile([P, NB], f32)
        nc.scalar.activation(
            out=t_exp[:], in_=t_abs[:], func=mybir.ActivationFunctionType.Exp,
            scale=-1.0,
        )
        t_ln = const.tile([P, NB], f32)
        nc.scalar.activation(
            out=t_ln[:], in_=t_exp[:], func=mybir.ActivationFunctionType.Ln,
            bias=1.0,
        )
        t_relu = const.tile([P, NB], f32)
        nc.scalar.activation(
            out=t_relu[:], in_=dm[:], func=mybir.ActivationFunctionType.Relu
        )

        acc = const.tile([P, 2 * NB], f32)
        nc.vector.tensor_tensor(
            out=acc[:, :NB], in0=t_ln[:], in1=t_relu[:], op=mybir.AluOpType.add
        )
        # reg masked + scaled (wval holds 0.5*1e-5 or 0)
        nc.vector.tensor_tensor(
            out=acc[:, NB:], in0=regs[:], in1=wv_sb[:], op=mybir.AluOpType.mult
        )

        # relation-embedding reg: cnt_b * 0.5*||r_b||^2 (cnt pre-scaled 1e-5)
        rsq = const.tile([NB, E], f32)
        nc.vector.tensor_tensor(
            out=rsq[:], in0=r_blk[:], in1=r_blk[:], op=mybir.AluOpType.mult
        )
        rr_col = const.tile([NB, 1], f32)
        nc.vector.reduce_sum(out=rr_col[:], in_=rsq[:], axis=mybir.AxisListType.X)
        rr_s = const.tile([NB, 1], f32)
        nc.vector.tensor_tensor(
            out=rr_s[:], in0=rr_col[:], in1=cnt_sb[:], op=mybir.AluOpType.mult
        )

        # total per-partition, then partition-reduce via ones matmul
        t_all = const.tile([P, 1], f32)
        nc.vector.reduce_sum(out=t_all[:], in_=acc[:], axis=mybir.AxisListType.X)
        nc.vector.tensor_tensor(
            out=t_all[:NB], in0=t_all[:NB], in1=rr_s[:], op=mybir.AluOpType.add
        )
        fin_ps = ps_mm.tile([1, 1], f32, tag="mp")
        nc.tensor.matmul(out=fin_ps[:], lhsT=t_all[:], rhs=ones_col[:], start=True, stop=True)
        fin_sb = const.tile([1, 1], f32)
        nc.scalar.copy(fin_sb[:], fin_ps[:])
        nc.sync.dma_start(out=out[:], in_=fin_sb[:])

    nc.compile()
    return nc


def _plan(h, r, pos_t, neg_t, relation_weight, relation_embed):
    """Sort by relation, pad to 128-row single-relation blocks, split 8 ways."""
    order = np.argsort(r, kind="stable")
    counts = np.bincount(r, minlength=N_REL)
    blocks = []
    pos = 0
    for k in range(N_REL):
        c = int(counts[k])
        ids = order[pos : pos + c]
        pos += c
        for s in range(0, c, P):
            blocks.append((k, ids[s : s + P]))
    nb = max(2, -(-len(blocks) // N_CORES))
    while len(blocks) < nb * N_CORES:
        blocks.append((0, np.empty(0, np.int64)))

    maps = []
    for c in range(N_CORES):
        core_blocks = blocks[c * nb : (c + 1) * nb]
        idx3 = np.zeros((P, nb, 3), np.int32)
        mb = np.full((P, nb), PAD_BIAS, np.float32)
        wv = np.zeros((P, nb), np.float32)
        cnt = np.zeros((nb, 1), np.float32)
        w_blk = np.zeros((P, nb, P), np.float32)
        r_blk = np.zeros((nb, E), np.float32)
        for b, (k, ids) in enumerate(core_blocks):
            n = len(ids)
            if n:
                idx3[:n, b, 0] = h[ids]
                idx3[:n, b, 1] = pos_t[ids]
                idx3[:n, b, 2] = neg_t[ids]
            mb[:n, b] = 0.0
            wv[:n, b] = 0.5 * LAM
            cnt[b, 0] = n * LAM
            w_blk[:, b, :] = relation_weight[k]
            r_blk[b, :] = relation_embed[k]
        maps.append(
            {
                "idx3": idx3.reshape(P, nb * 3),
                "mbias": mb,
                "wval": wv,
                "cnt": cnt,
                "w_all": np.ascontiguousarray(w_blk.reshape(P, nb * P)),
                "r_blk": r_blk,
                "lsel": np.kron(np.eye(nb, dtype=np.float32), np.ones((1, P), np.float32)),
            }
        )
    return nb, maps


def kernel(h, r, pos_t, neg_t, entity_embed, relation_embed, relation_weight):
    h = np.asarray(h).astype(np.int32)
    r = np.asarray(r).astype(np.int32)
    pos_t = np.asarray(pos_t).astype(np.int32)
    neg_t = np.asarray(neg_t).astype(np.int32)
    ent = np.ascontiguousarray(np.asarray(entity_embed, dtype=np.float32))
    re = np.ascontiguousarray(np.asarray(relation_embed, dtype=np.float32))
    rw = np.ascontiguousarray(np.asarray(relation_weight, dtype=np.float32))

    nb, maps = _plan(h, r, pos_t, neg_t, rw, re)
    if nb not in _cache:
        _cache[nb] = _build(nb)
    nc = _cache[nb]

    in_maps = [{"ent": ent, **maps[c]} for c in range(N_CORES)]

    if os.environ.get("KGE_SIM"):
        from concourse.bass_interp import CoreSim

        total = 0.0
        for c in range(N_CORES):
            sim = CoreSim(nc, trace=False)
            for name, arr in in_maps[c].items():
                sim.tensor(name)[:] = arr
            sim.simulate()
            total += float(sim.tensor("out")[0, 0])
        return np.float32(total / M)

    from concourse.bass_utils import run_bass_kernel_spmd

    res = run_bass_kernel_spmd(nc, in_maps, core_ids=list(range(N_CORES)))
    total = sum(float(res.results[c]["out"][0, 0]) for c in range(N_CORES))
    return np.float32(total / M)



# revision 2
# speedup vs baseline: 2.2894x; 2.2894x over previous
"""KGE (TransR-style) loss kernel for Trainium2, 8 NeuronCores.

Strategy (v2):
  - Host: sort the M=8192 triples by relation id, pad each relation's
    segment to 128-row blocks (~96 single-relation blocks), split evenly
    across 8 cores (one SPMD program). Per relation k the host precomputes
    G_k = W_k @ W_k^T and g2_k = 2*W_k @ r_k (both bf16), using the
    identity
      neg_score - pos_score = 0.5*rowdot(S@G, T) + S@g,
      S = Pt - Nt,  T = 2H - Pt - Nt
    so the device needs ONE matmul per block instead of four and no
    per-example ||.||^2 of the matmul outputs.
  - Device (per core, NB blocks, 4 gather chunks):
      * one fused multi-index indirect DMA per chunk gathers all h/pos/neg
        entity rows of 3 blocks into x_all  [Pool/SWDGE prep ~1us, vs 36
        separate gathers at ~1us each in v1]
      * per chunk: s' = Nt - Pt (DVE, bf16 out), u = Pt + Nt (GPSIMD),
        t' = -2H + u written into taug's 129-col strided layout (DVE);
        reg column = ACT Square+accum over the raw gathered chunk
      * per block: PE transpose of s' (bf16), ACT copy PSUM->SBUF,
        one matmul s'^T.T @ [G|g2] -> Z (PSUM f32), then ONE fused DVE
        tensor_tensor_reduce: dm_b = 0.5*sum(Z*[t'|-1]) + mbias
      * tail: stable softplus decomposition on dm [128,NB], reg scaled by
        0.5*lambda, free-dim reduce + ones-matmul partition reduce -> one
        f32 per core
  - Host adds the relation-embedding reg term and subtracts the known
    contribution of padding rows (they gather entity row 0), then /M.
"""

import os
from contextlib import ExitStack

import numpy as np
import ml_dtypes

import concourse.bass as bass
import concourse.tile as tile
from concourse import bacc, mybir
from concourse.masks import make_identity

M = 8192
E = 128
C = E + 1  # G columns + g2 column
N_ENT = 500000
N_REL = 64
LAM = 1e-5
P = 128
N_CORES = 8
NCH = 4  # gather chunks per core
PAD_BIAS = -30000.0

f32 = mybir.dt.float32
bf16 = mybir.dt.bfloat16
i32 = mybir.dt.int32

_cache = {}


def _build(NB: int):
    """Build + compile the single-core SPMD program for NB blocks/core."""
    assert NB % NCH == 0
    BPC = NB // NCH  # blocks per chunk

    nc = bacc.Bacc(
        "TRN2",
        target_bir_lowering=False,
        debug=False,
        num_devices=N_CORES,
    )

    ent = nc.dram_tensor("ent", (N_ENT, E), f32, kind="ExternalInput").ap()
    idx = nc.dram_tensor("idx", (P, NB * 3), i32, kind="ExternalInput").ap()
    mbias = nc.dram_tensor("mbias", (P, NB), f32, kind="ExternalInput").ap()
    ggd = nc.dram_tensor("gg", (P, NB * C), bf16, kind="ExternalInput").ap()
    out = nc.dram_tensor("out", (1, 1), f32, kind="ExternalOutput").ap()

    with tile.TileContext(nc) as tc, ExitStack() as ctx:
        const = ctx.enter_context(tc.tile_pool(name="const", bufs=1))
        up = ctx.enter_context(tc.tile_pool(name="up", bufs=2))
        sbp = ctx.enter_context(tc.tile_pool(name="sbp", bufs=3))
        scrp = ctx.enter_context(tc.tile_pool(name="scrp", bufs=3))
        xsqp = ctx.enter_context(tc.tile_pool(name="xsqp", bufs=2))
        stp = ctx.enter_context(tc.tile_pool(name="stp", bufs=3, space="PSUM"))
        zp = ctx.enter_context(tc.tile_pool(name="zp", bufs=3, space="PSUM"))
        finp = ctx.enter_context(tc.tile_pool(name="finp", bufs=1, space="PSUM"))

        # constants (Pool work up front, fills the idx-DMA wait)
        iden_bf = const.tile([P, P], bf16)
        make_identity(nc, iden_bf[:])
        ones_col = const.tile([P, 1], f32)
        nc.gpsimd.memset(ones_col[:], 1.0)
        taug = const.tile([P, NB * C], f32)
        nc.gpsimd.memset(taug[:], -1.0)

        idx_sb = const.tile([P, NB * 3], i32)
        nc.sync.dma_start(out=idx_sb[:], in_=idx[:])
        mb_sb = const.tile([P, NB], f32)
        nc.sync.dma_start(out=mb_sb[:], in_=mbias[:])
        gg_sb = const.tile([P, NB * C], bf16)
        half = (NB // 2) * C
        nc.sync.dma_start(out=gg_sb[:, :half], in_=ggd[:, :half])
        nc.sync.dma_start(out=gg_sb[:, half:], in_=ggd[:, half:])

        x_all = const.tile([P, NB * 3 * E], f32)
        s_all = const.tile([P, NB * E], bf16)
        dmcols = const.tile([P, NB], f32)
        regc = const.tile([P, NCH], f32)

        CW = BPC * 3 * E  # x columns per chunk
        for c in range(NCH):
            nc.gpsimd.indirect_dma_start(
                out=x_all[:, c * CW : (c + 1) * CW],
                out_offset=None,
                in_=ent[:],
                in_offset=bass.IndirectOffsetOnAxis(
                    ap=idx_sb[:, c * BPC * 3 : (c + 1) * BPC * 3], axis=0
                ),
            )

        for c in range(NCH):
            xc = x_all[:, c * CW : (c + 1) * CW].rearrange(
                "p (b t e) -> p t b e", b=BPC, t=3, e=E
            )
            hch = xc[:, 0]
            pch = xc[:, 1]
            nch = xc[:, 2]

            # s' = Nt - Pt  (= -S), bf16 for fast PE transpose/matmul
            sv = s_all[:, c * BPC * E : (c + 1) * BPC * E].rearrange(
                "p (b e) -> p b e", b=BPC, e=E
            )
            nc.vector.tensor_tensor(
                out=sv, in0=nch, in1=pch, op=mybir.AluOpType.subtract
            )

            # u = Pt + Nt on GPSIMD (Pool is idle after the gather preps)
            u = up.tile([P, BPC * E], f32, tag="u")
            uv = u[:].rearrange("p (b e) -> p b e", b=BPC, e=E)
            nc.gpsimd.tensor_tensor(out=uv, in0=pch, in1=nch, op=mybir.AluOpType.add)

            # t' = -2H + u  (= -T) into taug's [.. 128 cols of each 129 ..]
            tv = taug[:, c * BPC * C : (c + 1) * BPC * C].rearrange(
                "p (b c1) -> p b c1", b=BPC, c1=C
            )[:, :, 0:E]
            nc.vector.scalar_tensor_tensor(
                out=tv, in0=hch, scalar=-2.0, in1=uv,
                op0=mybir.AluOpType.mult, op1=mybir.AluOpType.add,
            )

            # raw reg column for this chunk (pads gather ent[0]; corrected on host)
            xsq = xsqp.tile([P, CW], f32, tag="xsq")
            nc.scalar.activation(
                out=xsq[:], in_=x_all[:, c * CW : (c + 1) * CW],
                func=mybir.ActivationFunctionType.Square,
                accum_out=regc[:, c : c + 1],
            )

            for b in range(c * BPC, (c + 1) * BPC):
                st_ps = stp.tile([P, P], bf16, tag="st")
                nc.tensor.transpose(
                    out=st_ps[:], in_=s_all[:, b * E : (b + 1) * E],
                    identity=iden_bf[:],
                )
                st_sb = sbp.tile([P, P], bf16, tag="stsb")
                nc.scalar.copy(st_sb[:], st_ps[:])
                z_ps = zp.tile([P, C], f32, tag="z")
                nc.tensor.matmul(
                    out=z_ps[:], lhsT=st_sb[:], rhs=gg_sb[:, b * C : (b + 1) * C],
                    start=True, stop=True,
                )
                scr = scrp.tile([P, C], f32, tag="scr")
                nc.vector.tensor_tensor_reduce(
                    out=scr[:], in0=z_ps[:], in1=taug[:, b * C : (b + 1) * C],
                    scale=0.5, scalar=mb_sb[:, b : b + 1],
                    op0=mybir.AluOpType.mult, op1=mybir.AluOpType.add,
                    accum_out=dmcols[:, b : b + 1],
                )

        # ---- batched tail ----
        # softplus(y) = relu(y) + ln(1 + exp(-|y|))
        t_abs = const.tile([P, NB], f32)
        nc.scalar.activation(
            out=t_abs[:], in_=dmcols[:], func=mybir.ActivationFunctionType.Abs
        )
        t_exp = const.tile([P, NB], f32)
        nc.scalar.activation(
            out=t_exp[:], in_=t_abs[:], func=mybir.ActivationFunctionType.Exp,
            scale=-1.0,
        )
        t_ln = const.tile([P, NB], f32)
        nc.scalar.activation(
            out=t_ln[:], in_=t_exp[:], func=mybir.ActivationFunctionType.Ln,
            bias=1.0,
        )
        t_relu = const.tile([P, NB], f32)
        nc.scalar.activation(
            out=t_relu[:], in_=dmcols[:], func=mybir.ActivationFunctionType.Relu
        )

        acc = const.tile([P, NB + NCH], f32)
        nc.vector.tensor_tensor(
            out=acc[:, :NB], in0=t_ln[:], in1=t_relu[:], op=mybir.AluOpType.add
        )
        nc.vector.tensor_scalar_mul(
            out=acc[:, NB:], in0=regc[:], scalar1=0.5 * LAM
        )
        t_all = const.tile([P, 1], f32)
        nc.vector.reduce_sum(out=t_all[:], in_=acc[:], axis=mybir.AxisListType.X)

        fin_ps = finp.tile([1, 1], f32, tag="fin")
        nc.tensor.matmul(
            out=fin_ps[:], lhsT=t_all[:], rhs=ones_col[:], start=True, stop=True
        )
        fin_sb = const.tile([1, 1], f32)
        nc.scalar.copy(fin_sb[:], fin_ps[:])
        nc.sync.dma_start(out=out[:], in_=fin_sb[:])

    nc.compile()
    return nc


def _plan(h, r, pos_t, neg_t, relation_weight, relation_embed):
    """Sort by relation, pad to 128-row single-relation blocks, split 8 ways."""
    order = np.argsort(r, kind="stable")
    counts = np.bincount(r, minlength=N_REL)
    blocks = []
    pos = 0
    for k in range(N_REL):
        c = int(counts[k])
        ids = order[pos : pos + c]
        pos += c
        for s in range(0, c, P):
            blocks.append((k, ids[s : s + P]))
    nb = -(-len(blocks) // N_CORES)
    nb = -(-nb // NCH) * NCH  # multiple of NCH chunks
    while len(blocks) < nb * N_CORES:
        blocks.append((0, np.empty(0, np.int64)))

    # per-relation [G_k | 2*W_k@r_k] in bf16
    gg_rel = np.zeros((N_REL, E, C), np.float32)
    gg_rel[:, :, :E] = np.einsum(
        "ker,kfr->kef", relation_weight, relation_weight, optimize=True
    )
    gg_rel[:, :, E] = 2.0 * np.einsum("ker,kr->ke", relation_weight, relation_embed)
    gg_rel = gg_rel.astype(ml_dtypes.bfloat16)

    maps = []
    pad_slots = 0
    for c in range(N_CORES):
        core_blocks = blocks[c * nb : (c + 1) * nb]
        idx3 = np.zeros((P, nb, 3), np.int32)
        mb = np.full((P, nb), PAD_BIAS, np.float32)
        gg = np.zeros((P, nb, C), ml_dtypes.bfloat16)
        for b, (k, ids) in enumerate(core_blocks):
            n = len(ids)
            if n:
                idx3[:n, b, 0] = h[ids]
                idx3[:n, b, 1] = pos_t[ids]
                idx3[:n, b, 2] = neg_t[ids]
                gg[:, b, :] = gg_rel[k]
            mb[:n, b] = 0.0
            pad_slots += 3 * (P - n)
        maps.append(
            {
                "idx": np.ascontiguousarray(idx3.reshape(P, nb * 3)),
                "mbias": mb,
                "gg": np.ascontiguousarray(gg.reshape(P, nb * C)),
            }
        )
    return nb, maps, counts, pad_slots


def kernel(h, r, pos_t, neg_t, entity_embed, relation_embed, relation_weight):
    h = np.asarray(h).astype(np.int32)
    r = np.asarray(r).astype(np.int32)
    pos_t = np.asarray(pos_t).astype(np.int32)
    neg_t = np.asarray(neg_t).astype(np.int32)
    ent = np.ascontiguousarray(np.asarray(entity_embed, dtype=np.float32))
    re = np.ascontiguousarray(np.asarray(relation_embed, dtype=np.float32))
    rw = np.ascontiguousarray(np.asarray(relation_weight, dtype=np.float32))

    nb, maps, counts, pad_slots = _plan(h, r, pos_t, neg_t, rw, re)
    if nb not in _cache:
        _cache[nb] = _build(nb)
    nc = _cache[nb]

    # host-side closed-form corrections (tiny, relation-table/O(E) work):
    # relation-embedding reg + removal of pad rows' gathered ent[0] reg
    r_norms = np.sum(re.astype(np.float64) ** 2, axis=1)
    r_reg = 0.5 * LAM * float(np.dot(counts.astype(np.float64), r_norms))
    ent0 = float(np.sum(ent[0].astype(np.float64) ** 2))
    pad_corr = 0.5 * LAM * pad_slots * ent0

    in_maps = [{"ent": ent, **maps[c]} for c in range(N_CORES)]

    if os.environ.get("KGE_SIM"):
        from concourse.bass_interp import CoreSim

        total = 0.0
        for c in range(N_CORES):
            sim = CoreSim(nc, trace=False)
            for name, arr in in_maps[c].items():
                sim.tensor(name)[:] = arr
            sim.simulate()
            total += float(sim.tensor("out")[0, 0])
        return np.float32((total + r_reg - pad_corr) / M)

    from concourse.bass_utils import run_bass_kernel_spmd

    res = run_bass_kernel_spmd(nc, in_maps, core_ids=list(range(N_CORES)))
    total = sum(float(res.results[c]["out"][0, 0]) for c in range(N_CORES))
    return np.float32((total + r_reg - pad_corr) / M)
